# revision 19
# baseline (speedup 1.0000x reference)
"""Masked attention kernel for Trainium2, row-parallel over 8 NeuronCores.

Reference computation (per problem):
    S   = (Q @ K^T) / sqrt(D)          [NQ, NK]
    S   = where(m, S, -1e30)
    P   = softmax(S, axis=-1)
    out = P @ V                        [NQ, D]

Strategy:
  * Shard queries across 8 cores (1024 rows each); K/V/mask-columns replicated
    or sliced appropriately. No collectives.
  * Scores are computed TRANSPOSED on-chip: S_T[k, q] = sum_d K[k,d] * Qs[q,d]
    with Qs = Q/sqrt(D) pre-scaled on host. This makes the second matmul
    (P^T as lhsT, V as rhs) transpose-free.
  * Softmax without max-subtraction (scores are ~N(0,1); exp is safe in f32),
    multiplying by the 0/1 mask after exp.
  * Denominator for free: V is extended with a ones column, so PSUM column 256
    accumulates sum_k P[q,k]; normalize with reciprocal + per-partition scale.
  * bf16 matmul inputs (f32 PSUM accumulation), f32 output.
"""

import os
import sys

import numpy as np

sys.path.insert(0, "/opt/trn_rl_repo")

import ml_dtypes

NQ, NK, D = 8192, 8192, 256
NCORES = 8
QSH = NQ // NCORES          # 1024 queries per core
P = 128
KT_TILES = NK // P          # 64 key tiles
QB = 512                    # q-block (matmul moving free dim)
NQB = QSH // QB             # 2 q-blocks per core
VE = D + 1                  # V extended with ones column

_STATE = {}
LAST_RESULTS = None
TRACE = bool(os.environ.get("BASS_TRACE"))


def _build():
    import concourse.tile as tile
    from concourse import bacc, mybir

    bf16 = mybir.dt.bfloat16
    f32 = mybir.dt.float32
    u8 = mybir.dt.uint8

    nc = bacc.Bacc("TRN2", debug=False, enable_asserts=False, num_devices=NCORES)

    # All big inputs are partition-major: [128, KT_TILES, free] so chunked
    # DMAs move large contiguous per-partition ranges.
    kt_d = nc.dram_tensor("KT", [P, KT_TILES, 2 * P], bf16, kind="ExternalInput").ap()
    vt_d = nc.dram_tensor("VT", [P, KT_TILES, VE], bf16, kind="ExternalInput").ap()
    qt_d = nc.dram_tensor("QT", [P, 2, QSH], bf16, kind="ExternalInput").ap()
    mt_d = nc.dram_tensor("MT", [P, KT_TILES, QSH], u8, kind="ExternalInput").ap()
    out_d = nc.dram_tensor("out", [QSH, D], f32, kind="ExternalOutput").ap()

    Exp = mybir.ActivationFunctionType.Exp
    mult = mybir.AluOpType.mult

    with tile.TileContext(nc) as tc:
        with (
            tc.tile_pool(name="singles", bufs=1) as singles,
            tc.tile_pool(name="pp", bufs=8) as pp,
            tc.tile_pool(name="outp", bufs=6) as outp,
            tc.tile_pool(name="smallp", bufs=4) as smallp,
            tc.tile_pool(name="spsum", bufs=4, space="PSUM") as spsum,
            tc.tile_pool(name="opsum", bufs=1, space="PSUM") as opsum,
        ):
            # Prewarm the ACT exp table so its ~2.7us load overlaps input DMA.
            warm = singles.tile([P, 1], f32)
            nc.vector.memset(warm, 0.0)
            warm2 = smallp.tile([P, 1], f32, tag="warm2")
            nc.scalar.activation(warm2, warm, Exp)

            # Warm the PE HAM clock gate (~3.4us of matmul activity) while the
            # input DMAs stream in, so the real k-loop starts at 2.4 GHz.
            wdummy = singles.tile([P, QB], bf16)
            nc.vector.memset(wdummy, 0.0)
            w_ps = spsum.tile([P, QB], f32, tag="s", name="w_ps")
            for i in range(8):
                nc.tensor.matmul(
                    w_ps, lhsT=wdummy[:, 0:P], rhs=wdummy,
                    start=(i == 0), stop=(i == 7),
                )

            qt_sb = singles.tile([P, 2, QSH], bf16)
            kt_sb = singles.tile([P, KT_TILES, 2 * P], bf16)
            vt_sb = singles.tile([P, KT_TILES, VE], bf16)
            mt_sb = singles.tile([P, KT_TILES, QSH], u8)

            # Chunked input DMAs, ordered by first-use time in the k-loop;
            # first chunks are small so the loop can start early.
            def kt_dma(a, b):
                nc.sync.dma_start(out=kt_sb[:, a:b, :], in_=kt_d[:, a:b, :])

            def vt_dma(a, b):
                nc.sync.dma_start(out=vt_sb[:, a:b, :], in_=vt_d[:, a:b, :])

            def mt_dma(a, b):
                nc.sync.dma_start(out=mt_sb[:, a:b, :], in_=mt_d[:, a:b, :])

            nc.sync.dma_start(out=qt_sb[:, :, 0:QB], in_=qt_d[:, :, 0:QB])
            groups = [(0, 2), (2, 4), (4, 8), (8, 12), (12, 16), (16, 24),
                      (24, 32), (32, 40), (40, 48), (48, 56), (56, 64)]
            for gi, (a, b) in enumerate(groups):
                kt_dma(a, b)
                vt_dma(a, b)
                mt_dma(a, b)
                if gi == 4:
                    # qb1 half of Q — not needed until the second pass.
                    nc.sync.dma_start(
                        out=qt_sb[:, :, QB:QSH], in_=qt_d[:, :, QB:QSH]
                    )

            for qb in range(NQB):
                o_ps = [
                    opsum.tile([P, VE], f32, tag=f"o{qs}", name=f"o_ps{qs}")
                    for qs in range(4)
                ]
                for t in range(KT_TILES):
                    s_ps = spsum.tile([P, QB], f32, tag="s")
                    nc.tensor.matmul(
                        s_ps,
                        lhsT=kt_sb[:, t, 0:P],
                        rhs=qt_sb[:, 0, qb * QB:(qb + 1) * QB],
                        start=True,
                        stop=False,
                    )
                    nc.tensor.matmul(
                        s_ps,
                        lhsT=kt_sb[:, t, P:2 * P],
                        rhs=qt_sb[:, 1, qb * QB:(qb + 1) * QB],
                        start=False,
                        stop=True,
                    )
                    p_sb = pp.tile([P, QB], bf16, tag="p")
                    nc.scalar.activation(p_sb, s_ps, Exp)
                    nc.vector.tensor_tensor(
                        p_sb, p_sb, mt_sb[:, t, qb * QB:(qb + 1) * QB], mult
                    )
                    for qs in range(4):
                        nc.tensor.matmul(
                            o_ps[qs],
                            lhsT=p_sb[:, qs * P:(qs + 1) * P],
                            rhs=vt_sb[:, t, :],
                            start=(t == 0),
                            stop=(t == KT_TILES - 1),
                        )
                for qs in range(4):
                    recip = smallp.tile([P, 1], f32, tag="recip")
                    nc.vector.reciprocal(recip, o_ps[qs][:, D:D + 1])
                    o_sb = outp.tile([P, D], f32, tag="osb")
                    if qs % 2 == 0:
                        nc.vector.tensor_scalar_mul(o_sb, o_ps[qs][:, 0:D], recip)
                    else:
                        # ACT does the other half so the epilogue runs on two
                        # engines in parallel.
                        nc.scalar.mul(o_sb, o_ps[qs][:, 0:D], recip)
                    row0 = qb * QB + qs * P
                    nc.sync.dma_start(out=out_d[row0:row0 + P, :], in_=o_sb)

    nc.compile()
    return nc


def _get_nc():
    if "nc" not in _STATE:
        _STATE["nc"] = _build()
    return _STATE["nc"]


def _prep_inputs(K, V, Q, m):
    bf16 = ml_dtypes.bfloat16
    scale = 1.0 / np.sqrt(np.float32(D))

    # KT[p, t, c*128+k] = K[t*128+k, c*128+p]   (p = d % 128, c = d // 128)
    kt = np.ascontiguousarray(
        K.astype(np.float32).reshape(KT_TILES, P, 2, P).transpose(3, 0, 2, 1)
    ).astype(bf16).reshape(P, KT_TILES, 2 * P)

    # VT[p, t, n] = V_ext[t*128+p, n]
    vt = np.ones((NK, VE), dtype=np.float32)
    vt[:, :D] = V
    vt = np.ascontiguousarray(
        vt.astype(bf16).reshape(KT_TILES, P, VE).transpose(1, 0, 2)
    )

    # QT[p, c, q] = Q_scaled[q, c*128+p]  (per-core slice of q)
    qs_all = (Q.astype(np.float32) * scale).T.astype(bf16)  # [D, NQ]
    mt_all = np.ascontiguousarray(m.astype(np.uint8).T)     # [NK, NQ]

    in_maps = []
    for c in range(NCORES):
        q0 = c * QSH
        qt_c = np.ascontiguousarray(
            qs_all[:, q0:q0 + QSH].reshape(2, P, QSH).transpose(1, 0, 2)
        )
        # MT[p, t, q] = m[q0 + q, t*128 + p]
        mt_c = np.ascontiguousarray(
            mt_all[:, q0:q0 + QSH].reshape(KT_TILES, P, QSH).transpose(1, 0, 2)
        )
        in_maps.append({"KT": kt, "VT": vt, "QT": qt_c, "MT": mt_c})
    return in_maps


def kernel(K, V, Q, m):
    global LAST_RESULTS
    from concourse.bass_utils import run_bass_kernel_spmd

    nc = _get_nc()
    in_maps = _prep_inputs(
        np.asarray(K), np.asarray(V), np.asarray(Q), np.asarray(m)
    )
    try:
        res = run_bass_kernel_spmd(
            nc, in_maps, core_ids=list(range(NCORES)), trace=TRACE
        )
    except (ImportError, ModuleNotFoundError):
        # Profiling hook unavailable in this environment — run untraced.
        os.environ.pop("BASS_TRACE", None)
        res = run_bass_kernel_spmd(
            nc, in_maps, core_ids=list(range(NCORES)), trace=False
        )
    LAST_RESULTS = res
    out = np.concatenate([res.results[c]["out"] for c in range(NCORES)], axis=0)
    return out.astype(np.float32)


# revision 29
# speedup vs baseline: 1.2078x; 1.2078x over previous
"""Masked attention kernel for Trainium2, row-parallel over 8 NeuronCores.

Reference computation (per problem):
    S   = (Q @ K^T) / sqrt(D)          [NQ, NK]
    S   = where(m, S, -1e30)
    P   = softmax(S, axis=-1)
    out = P @ V                        [NQ, D]

Strategy:
  * Shard queries across 8 cores (1024 rows each); K/V/mask-columns replicated
    or sliced appropriately. No collectives.
  * Scores are computed TRANSPOSED on-chip: S_T[k, q] = sum_d K[k,d] * Qs[q,d]
    with Qs = Q/sqrt(D) pre-scaled on host. This makes the second matmul
    (P^T as lhsT, V as rhs) transpose-free.
  * Softmax without max-subtraction (scores are ~N(0,1); exp is safe in f32),
    multiplying by the 0/1 mask after exp.
  * Denominator for free: V is extended with a ones column, so PSUM column 256
    accumulates sum_k P[q,k]; normalize with reciprocal + per-partition scale.
  * bf16 matmul inputs (f32 PSUM accumulation), f32 output.
"""

import os
import sys

import numpy as np

sys.path.insert(0, "/opt/trn_rl_repo")

import ml_dtypes

NQ, NK, D = 8192, 8192, 256
NCORES = 8
QSH = NQ // NCORES          # 1024 queries per core
P = 128
KT_TILES = NK // P          # 64 key tiles
QB = 512                    # q-block (matmul moving free dim)
NQB = QSH // QB             # 2 q-blocks per core
VE = D + 1                  # V extended with ones column

_STATE = {}
LAST_RESULTS = None
TRACE = bool(os.environ.get("BASS_TRACE"))


def _build():
    import concourse.tile as tile
    from concourse import bacc, mybir

    bf16 = mybir.dt.bfloat16
    f32 = mybir.dt.float32
    u8 = mybir.dt.uint8

    nc = bacc.Bacc("TRN2", debug=False, enable_asserts=False, num_devices=NCORES)

    # All big inputs are partition-major: [128, KT_TILES, free] so chunked
    # DMAs move large contiguous per-partition ranges.
    kt_d = nc.dram_tensor("KT", [P, KT_TILES, 2 * P], bf16, kind="ExternalInput").ap()
    vt_d = nc.dram_tensor("VT", [P, KT_TILES, VE], bf16, kind="ExternalInput").ap()
    qt_d = nc.dram_tensor("QT", [P, 2, QSH], bf16, kind="ExternalInput").ap()
    mt_d = nc.dram_tensor("MT", [P, KT_TILES, QSH], u8, kind="ExternalInput").ap()
    out_d = nc.dram_tensor("out", [QSH, D], f32, kind="ExternalOutput").ap()

    Exp = mybir.ActivationFunctionType.Exp
    mult = mybir.AluOpType.mult

    with tile.TileContext(nc) as tc:
        with (
            tc.tile_pool(name="singles", bufs=1) as singles,
            tc.tile_pool(name="pp", bufs=8) as pp,
            tc.tile_pool(name="outp", bufs=6) as outp,
            tc.tile_pool(name="smallp", bufs=4) as smallp,
            tc.tile_pool(name="spsum", bufs=4, space="PSUM") as spsum,
            tc.tile_pool(name="opsum", bufs=1, space="PSUM") as opsum,
        ):
            # Prewarm the ACT exp table so its ~2.7us load overlaps input DMA.
            warm = singles.tile([P, 1], f32)
            nc.vector.memset(warm, 0.0)
            warm2 = smallp.tile([P, 1], f32, tag="warm2")
            nc.scalar.activation(warm2, warm, Exp)

            # Warm the PE HAM clock gate (~3.4us of matmul activity) while the
            # input DMAs stream in, so the real k-loop starts at 2.4 GHz.
            # The dummy matmuls read a raw (untracked, uninitialized) SBUF
            # tensor so they have no dependencies and start right after the
            # prologue barrier.
            wdummy = nc.alloc_sbuf_tensor("wdummy", [P, QB], bf16).ap()
            w_ps = spsum.tile([P, QB], f32, tag="s", name="w_ps")
            NWARM = 11
            for i in range(NWARM):
                nc.tensor.matmul(
                    w_ps, lhsT=wdummy[:, 0:P], rhs=wdummy,
                    start=(i == 0), stop=(i == NWARM - 1),
                )

            qt_sb = singles.tile([P, 2, QSH], bf16)
            kt_sb = singles.tile([P, KT_TILES, 2 * P], bf16)
            vt_sb = singles.tile([P, KT_TILES, VE], bf16)
            mt_sb = singles.tile([P, KT_TILES, QSH], u8)

            # Chunked input DMAs, ordered by first-use time in the k-loop;
            # first chunks are small so the loop can start early.
            def kt_dma(a, b):
                nc.sync.dma_start(out=kt_sb[:, a:b, :], in_=kt_d[:, a:b, :])

            def vt_dma(a, b):
                nc.sync.dma_start(out=vt_sb[:, a:b, :], in_=vt_d[:, a:b, :])

            def mt_dma(a, b, eng=None):
                (eng or nc.sync).dma_start(
                    out=mt_sb[:, a:b, :], in_=mt_d[:, a:b, :]
                )

            nc.sync.dma_start(out=qt_sb[:, :, 0:QB], in_=qt_d[:, :, 0:QB])
            groups = [(0, 4), (4, 8), (8, 16), (16, 24), (24, 32),
                      (32, 40), (40, 48), (48, 56), (56, 64)]
            for gi, (a, b) in enumerate(groups):
                kt_dma(a, b)
                vt_dma(a, b)
                mt_dma(a, b)
                if gi == 3:
                    # qb1 half of Q — not needed until the second pass.
                    nc.sync.dma_start(
                        out=qt_sb[:, :, QB:QSH], in_=qt_d[:, :, QB:QSH]
                    )

            SKEW = 3  # mm1/exp/mask run this many k-tiles ahead of mm2

            for qb in range(NQB):
                o_ps = [
                    opsum.tile([P, VE], f32, tag=f"o{qs}", name=f"o_ps{qs}")
                    for qs in range(4)
                ]
                p_tiles = {}
                for tt in range(KT_TILES + SKEW):
                    if tt < KT_TILES:
                        t = tt
                        s_ps = spsum.tile([P, QB], f32, tag="s")
                        nc.tensor.matmul(
                            s_ps,
                            lhsT=kt_sb[:, t, 0:P],
                            rhs=qt_sb[:, 0, qb * QB:(qb + 1) * QB],
                            start=True,
                            stop=False,
                        )
                        nc.tensor.matmul(
                            s_ps,
                            lhsT=kt_sb[:, t, P:2 * P],
                            rhs=qt_sb[:, 1, qb * QB:(qb + 1) * QB],
                            start=False,
                            stop=True,
                        )
                        p_sb = pp.tile([P, QB], bf16, tag="p")
                        nc.scalar.activation(p_sb, s_ps, Exp)
                        nc.vector.tensor_tensor(
                            p_sb, p_sb, mt_sb[:, t, qb * QB:(qb + 1) * QB], mult
                        )
                        p_tiles[t] = p_sb
                    if tt >= SKEW:
                        t = tt - SKEW
                        p_sb = p_tiles.pop(t)
                        for qs in range(4):
                            nc.tensor.matmul(
                                o_ps[qs],
                                lhsT=p_sb[:, qs * P:(qs + 1) * P],
                                rhs=vt_sb[:, t, :],
                                start=(t == 0),
                                stop=(t == KT_TILES - 1),
                            )
                for qs in range(4):
                    recip = smallp.tile([P, 1], f32, tag="recip")
                    nc.vector.reciprocal(recip, o_ps[qs][:, D:D + 1])
                    o_sb = outp.tile([P, D], f32, tag="osb")
                    if qs % 2 == 0:
                        nc.vector.tensor_scalar_mul(o_sb, o_ps[qs][:, 0:D], recip)
                    else:
                        # ACT does the other half so the epilogue runs on two
                        # engines in parallel.
                        nc.scalar.mul(o_sb, o_ps[qs][:, 0:D], recip)
                    row0 = qb * QB + qs * P
                    nc.sync.dma_start(out=out_d[row0:row0 + P, :], in_=o_sb)

    nc.compile()
    return nc


def _get_nc():
    if "nc" not in _STATE:
        _STATE["nc"] = _build()
    return _STATE["nc"]


def _prep_inputs(K, V, Q, m):
    bf16 = ml_dtypes.bfloat16
    scale = 1.0 / np.sqrt(np.float32(D))

    # KT[p, t, c*128+k] = K[t*128+k, c*128+p]   (p = d % 128, c = d // 128)
    kt = np.ascontiguousarray(
        K.astype(np.float32).reshape(KT_TILES, P, 2, P).transpose(3, 0, 2, 1)
    ).astype(bf16).reshape(P, KT_TILES, 2 * P)

    # VT[p, t, n] = V_ext[t*128+p, n]
    vt = np.ones((NK, VE), dtype=np.float32)
    vt[:, :D] = V
    vt = np.ascontiguousarray(
        vt.astype(bf16).reshape(KT_TILES, P, VE).transpose(1, 0, 2)
    )

    # QT[p, c, q] = Q_scaled[q, c*128+p]  (per-core slice of q)
    qs_all = (Q.astype(np.float32) * scale).T.astype(bf16)  # [D, NQ]
    mt_all = np.ascontiguousarray(m.astype(np.uint8).T)     # [NK, NQ]

    in_maps = []
    for c in range(NCORES):
        q0 = c * QSH
        qt_c = np.ascontiguousarray(
            qs_all[:, q0:q0 + QSH].reshape(2, P, QSH).transpose(1, 0, 2)
        )
        # MT[p, t, q] = m[q0 + q, t*128 + p]
        mt_c = np.ascontiguousarray(
            mt_all[:, q0:q0 + QSH].reshape(KT_TILES, P, QSH).transpose(1, 0, 2)
        )
        in_maps.append({"KT": kt, "VT": vt, "QT": qt_c, "MT": mt_c})
    return in_maps


def kernel(K, V, Q, m):
    global LAST_RESULTS
    from concourse.bass_utils import run_bass_kernel_spmd

    nc = _get_nc()
    in_maps = _prep_inputs(
        np.asarray(K), np.asarray(V), np.asarray(Q), np.asarray(m)
    )
    try:
        res = run_bass_kernel_spmd(
            nc, in_maps, core_ids=list(range(NCORES)), trace=TRACE
        )
    except Exception:
        # Profiling hook unavailable or a transient runtime failure — retry
        # once, untraced.
        os.environ.pop("BASS_TRACE", None)
        res = run_bass_kernel_spmd(
            nc, in_maps, core_ids=list(range(NCORES)), trace=False
        )
    LAST_RESULTS = res
    out = np.concatenate([res.results[c]["out"] for c in range(NCORES)], axis=0)
    return out.astype(np.float32)
